# revision 9
# baseline (speedup 1.0000x reference)
"""Trainium2 Bass kernel for nn_CP_LIF (LIF neurons, softplus-parameterized
tau / soft-reset, surrogate-gradient spike forward = hard threshold).

Reference semantics per step (v-space, fp32):
    v   = alpha*v + (1-alpha)*x_t          # alpha = exp(-1/tau), per-neuron
    s   = (v - 1 > 0)                      # forward value of surrogate spike
    v   = v - s*r                          # soft reset, per-neuron r

Device math (P-space): Z := (v' - 1)/(1-alpha) + 1, so the threshold is the
constant 1 and the input current is RAW x (no per-neuron input scaling):
    Z_{t+1} = ((Z_t - 1) - (Z_t > 1)*C1) * C0 + x_{t+1}
    s_t     = (Z_t > 1)
with per-neuron constants C0 = alpha, C1 = 1/bprime (bprime = (1-alpha)/r).

The key trick: a single custom DVE instruction evaluates a whole GROUP of
timesteps of the recurrence via in-instruction self-feedback. Per neuron
chunk, the Z tile holds 1+KB blocks of 128 batch columns:
[Z_t0 | x_{t0+1} .. x_{t0+KB}]. The op's in0 AP covers blocks 0..KB-1 while
its out (and in1) AP covers blocks 1..KB — the DVE streams the free
dimension in order at 1 elem/cycle, so the output block written ~120 cycles
earlier is re-read as the next step's state (verified bit-exact on HW).
x is overwritten by Z in place.

Engines per group (n-major layout, 4 chunks of 128 neurons):
    DVE   : 4 chunk ops, FD = KB*128 each    (the entire recurrence)
    GPSIMD: 4 tiny boundary copies Z_tKB -> next tile block 0 (hidden)
    ACT   : spikes = Sigmoid(1e30*Z - 1e30) -> uint8, all 4 chunks at once
    PE    : completely idle
    DMA   : x in on the SP HWDGE ring (prefetched 2 groups ahead),
            spikes out on the ACT HWDGE ring

Group sizes ramp up (2,3,5,10,10,...) so the pipeline fill is one small DMA
instead of a full-size group. Steady state is HBM-bandwidth-bound
(~330 KB per step per core vs ~358 GB/s per-NeuronCore limit).

Sharding: neurons split 8 ways (512/core), batch full on every core; no
cross-core communication. Measured ~0 flipped spikes vs the fp32 CPU
reference on the full 100x128x4096 problem.
"""

import sys

import numpy as np

if "/opt/trn_rl_repo" not in sys.path:
    sys.path.insert(0, "/opt/trn_rl_repo")

T, B, N = 100, 128, 4096
NCORES = 8
NLOC = N // NCORES          # 512 neurons per core
NCH = NLOC // 128           # 4 partition-chunks of the neuron dim
BLK = 128                   # batch block width (one timestep column block)

DT = 1.0
V_TH = 1.0
TAU_MIN = 1e-3
R_MIN = 1e-6

KB = 10                     # steady-state timesteps per DVE instruction
RAMP_UP = (4, 6)            # pipeline-fill group sizes
RAMP_DN = (5, 3, 2)         # pipeline-drain group sizes
PF = 1                      # x-DMA prefetch depth (groups ahead)

_NC_CACHE = {}
_LIF_OP = None


def group_sizes(n_steps):
    """Ramp-up + steady + ramp-down group sizes summing to n_steps."""
    up, dn = list(RAMP_UP), list(RAMP_DN)
    rem = n_steps - sum(up) - sum(dn)
    if rem < 0 or rem % KB:
        # fallback: uniform KB groups with a remainder group
        gs = [KB] * (n_steps // KB)
        if n_steps % KB:
            gs.append(n_steps % KB)
        return gs
    return up + [KB] * (rem // KB) + dn


def _register_lif_op():
    """Custom DVE op: out = ((in0 - 1) - (in0 > 1)*C1)*C0 + in1.

    With in0 = Z_t (prev state block), in1 = x_{t+1}, C0 = alpha (per
    partition), C1 = 1/bprime (per partition) this computes Z_{t+1}; the
    in0/out APs overlap shifted by one 128-col block so one instruction
    evaluates KB serial timesteps.
    """
    global _LIF_OP
    if _LIF_OP is not None:
        return _LIF_OP
    import concourse.dve_ops as dve_ops
    from concourse.dve_ops import DveOp, OPS, CUSTOM_DVE_SPECS, _SUB_OPCODE_FOR_NAME
    from concourse.dve_spec import Spec, Src0, Src1, C0, C1, One, lower
    from concourse.dve_uop import DveOpSpec

    name = "LIF_STREAM_ANT"
    if name in _SUB_OPCODE_FOR_NAME:
        _LIF_OP = next(op for op in OPS if op.name == name)
        return _LIF_OP

    z = Src0
    s = z > One
    spec = Spec(
        body=((z - One) - s * C1) * C0 + Src1,
        reference=lambda in0, in1, s0, s1, imm2: (
            ((in0 - 1.0) - (in0 > 1.0).astype(np.float32) * s1) * s0 + in1
        ).astype(np.float32),
    )
    row = dve_ops._CUSTOM_DVE_ROW_BASE + len(OPS)
    assert row < 0x20
    shas = {}
    for ver in ("v3", "v4"):
        tmp = DveOpSpec(name=name, opcode=row, uops=lower(spec, ver=ver),
                        rd1_en=True)
        shas[ver] = tmp.sha(ver)
    op = DveOp(name, spec, subdim=False, uops_sha=shas)
    OPS.append(op)
    CUSTOM_DVE_SPECS[name] = spec
    _SUB_OPCODE_FOR_NAME[name] = row
    _LIF_OP = op
    return op


def _build_nc(n_steps=T):
    import concourse.bacc as bacc
    import concourse.tile as tile
    from concourse import mybir

    f32 = mybir.dt.float32
    u8 = mybir.dt.uint8

    lif_op = _register_lif_op()

    nc = bacc.Bacc("TRN2", target_bir_lowering=False, debug=False)

    xf = nc.dram_tensor("xf", [128, NCH * n_steps * BLK], f32,
                        kind="ExternalInput").ap()
    z0 = nc.dram_tensor("z0", [128, NCH * BLK], f32, kind="ExternalInput").ap()
    alpha4 = nc.dram_tensor("alpha4", [128, NCH], f32, kind="ExternalInput").ap()
    rinv4 = nc.dram_tensor("rinv4", [128, NCH], f32, kind="ExternalInput").ap()
    sf = nc.dram_tensor("sf", [128, NCH * n_steps * BLK], u8,
                        kind="ExternalOutput").ap()

    _emit(nc, tile, mybir, lif_op, xf, z0, alpha4, rinv4, sf, n_steps, reps=1)

    nc.compile()
    return nc


def _emit(nc, tile, mybir, lif_op, xf, z0, alpha4, rinv4, sf, n_steps, reps=1):
    f32 = mybir.dt.float32
    u8 = mybir.dt.uint8
    from contextlib import nullcontext

    gs = group_sizes(n_steps)
    G = len(gs)
    t0s = [sum(gs[:i]) for i in range(G)]   # start step of each group
    SECMAX = (KB + 1) * BLK

    xf3 = xf.rearrange("p (c t) -> p c t", c=NCH)    # t in units of BLK cols
    sf3 = sf.rearrange("p (c t) -> p c t", c=NCH)

    with tile.TileContext(nc) as tc:
        with (
            tc.tile_pool(name="const", bufs=1) as const,
            tc.tile_pool(name="zp", bufs=PF + 3) as zpool,
            tc.tile_pool(name="sp", bufs=3) as spool,
        ):
            a_t = const.tile([128, NCH], f32)
            nc.sync.dma_start(a_t[:], alpha4)
            ri_t = const.tile([128, NCH], f32)
            nc.sync.dma_start(ri_t[:], rinv4)
            nb_t = const.tile([128, 1], f32)
            nc.gpsimd.memset(nb_t[:], -1.0e30)
            guard_t = const.tile([128, NCH], f32)

            rep_cm = tc.For_i(0, reps, 1) if reps > 1 else nullcontext()
            with rep_cm:
                zts = {}

                def alloc_and_fetch(i):
                    kb = gs[i]
                    sec = (kb + 1) * BLK
                    zt = zpool.tile([128, NCH * SECMAX], f32)
                    zts[i] = zt
                    dst = zt[:].rearrange("p (c y) -> p c y", c=NCH)[
                        :, :, BLK:sec
                    ]
                    src = xf3[:, :, t0s[i] * BLK:(t0s[i] + kb) * BLK]
                    nc.sync.dma_start(dst, src)

                # prologue: prefetch x for the first PF+1 groups, seed state
                for i in range(min(PF + 1, G)):
                    alloc_and_fetch(i)
                dst = zts[0][:].rearrange("p (c y) -> p c y", c=NCH)[:, :, :BLK]
                nc.sync.dma_start(dst, z0.rearrange("p (c b) -> p c b", c=NCH))

                for g in range(G):
                    zt = zts[g]
                    kb = gs[g]
                    sec = (kb + 1) * BLK

                    # Dependency guard: the last x block of each chunk is read
                    # only via in1, which aliases the op's own out region —
                    # Tile does not derive the DMA->op edge for it. This tiny
                    # read of one column per chunk forces the wait; the real
                    # ops follow in DVE program order.
                    nc.vector.tensor_copy(
                        guard_t[:].rearrange("p (c y) -> p c y", y=1),
                        zt[:].rearrange("p (c y) -> p c y", c=NCH)[
                            :, :, kb * BLK:kb * BLK + 1
                        ],
                    )

                    # the recurrence: one DVE op per chunk, kb steps each,
                    # in-instruction self-feedback across the kb blocks
                    for c in range(NCH):
                        base = c * SECMAX
                        nc.vector._custom_dve(
                            lif_op,
                            out=zt[:, base + BLK:base + sec],
                            in0=zt[:, base:base + kb * BLK],
                            in1=zt[:, base + BLK:base + sec],
                            s0=a_t[:, c:c + 1],
                            s1=ri_t[:, c:c + 1],
                        )

                    # state boundary into the next tile (GPSIMD: off both the
                    # DVE chain and the ACT queue, hidden under the next ops)
                    if g + 1 < G:
                        znext = zts[g + 1]
                        for c in range(NCH):
                            nc.gpsimd.tensor_copy(
                                znext[:, c * SECMAX:c * SECMAX + BLK],
                                zt[:, c * SECMAX + kb * BLK:c * SECMAX + sec],
                            )

                    # spikes: s = (Z > 1) as exact 0/1 u8
                    st = spool.tile([128, NCH * KB * BLK], u8)
                    nc.scalar.activation(
                        st[:].rearrange("p (c y) -> p c y", c=NCH)[
                            :, :, :kb * BLK
                        ],
                        zt[:].rearrange("p (c y) -> p c y", c=NCH)[
                            :, :, BLK:sec
                        ],
                        mybir.ActivationFunctionType.Sigmoid,
                        bias=nb_t[:, 0:1],
                        scale=1.0e30,
                    )
                    # spikes out on the ACT HWDGE ring (overlaps the x-in ring)
                    nc.scalar.dma_start(
                        sf3[:, :, t0s[g] * BLK:(t0s[g] + kb) * BLK],
                        st[:].rearrange("p (c y) -> p c y", c=NCH)[
                            :, :, :kb * BLK
                        ],
                    )

                    if g + PF + 1 < G:
                        alloc_and_fetch(g + PF + 1)
                    del zts[g]


def _get_nc(n_steps=T):
    if n_steps not in _NC_CACHE:
        _NC_CACHE[n_steps] = _build_nc(n_steps)
    return _NC_CACHE[n_steps]


def _derive_params(tau_raw, r_raw):
    """Per-neuron constants, fp32 softplus path matching jax CPU exactly."""
    tr = np.asarray(tau_raw, dtype=np.float32)
    rr = np.asarray(r_raw, dtype=np.float32)
    tau = np.logaddexp(np.float32(0.0), tr).astype(np.float32) + np.float32(TAU_MIN)
    alpha = np.exp(-np.float32(DT) / tau).astype(np.float32)
    r = np.logaddexp(np.float32(0.0), rr).astype(np.float32) + np.float32(R_MIN)
    beta = np.float32(1.0) - alpha
    # C1 = 1/bprime = r/beta; z0 = (1 - 1/beta)/alpha + 1 in f64 then f32
    c1 = (r.astype(np.float64) / beta.astype(np.float64)).astype(np.float32)
    z0 = (
        (1.0 - 1.0 / beta.astype(np.float64)) / alpha.astype(np.float64) + 1.0
    ).astype(np.float32)
    return alpha, c1, z0


def _core_inputs(x, alpha, c1, z0, core, n_steps):
    sl = slice(core * NLOC, (core + 1) * NLOC)
    # x[:, :, sl] is [T, B, 512]; device wants [p, (c, t, b)] flat
    xs = x[:n_steps, :, sl].reshape(n_steps, B, NCH, 128)
    xfl = np.ascontiguousarray(xs.transpose(3, 2, 0, 1), dtype=np.float32)
    xfl = xfl.reshape(128, NCH * n_steps * BLK)

    a4 = np.ascontiguousarray(alpha[sl].reshape(NCH, 128).T, dtype=np.float32)
    r4 = np.ascontiguousarray(c1[sl].reshape(NCH, 128).T, dtype=np.float32)
    z0l = z0[sl].reshape(NCH, 128).T      # [p, c]
    z0b = np.ascontiguousarray(
        np.broadcast_to(z0l[:, :, None], (128, NCH, BLK)), dtype=np.float32
    ).reshape(128, NCH * BLK)
    return {"xf": xfl, "z0": z0b, "alpha4": a4, "rinv4": r4}


def _run(x, tau_raw, r_raw, n_steps=T, trace=False, **run_kwargs):
    from concourse.bass_utils import run_bass_kernel_spmd

    alpha, c1, z0 = _derive_params(tau_raw, r_raw)
    in_maps = [
        _core_inputs(x, alpha, c1, z0, c, n_steps) for c in range(NCORES)
    ]
    nc = _get_nc(n_steps)
    res = run_bass_kernel_spmd(
        nc, in_maps, core_ids=list(range(NCORES)), trace=trace, **run_kwargs
    )
    shards = []
    for c in range(NCORES):
        sfl = res.results[c]["sf"].reshape(128, NCH, n_steps, BLK)
        # [p, c, t, b] -> [t, b, n_local = c*128 + p]
        sc = sfl.transpose(2, 3, 1, 0).reshape(n_steps, B, NLOC)
        shards.append(sc)
    out = np.concatenate(shards, axis=-1).astype(np.float32)
    return out, res


def kernel(x, tau_raw, r_raw):
    x = np.asarray(x, dtype=np.float32)
    tau_raw = np.asarray(tau_raw, dtype=np.float32)
    r_raw = np.asarray(r_raw, dtype=np.float32)
    last = None
    for attempt in range(3):
        try:
            out, _ = _run(x, tau_raw, r_raw)
            return out
        except Exception as e:  # transient NRT device errors observed rarely
            last = e
            import time as _time

            _time.sleep(2.0 * (attempt + 1))
    raise last


# revision 10
# speedup vs baseline: 1.0053x; 1.0053x over previous
"""Trainium2 Bass kernel for nn_CP_LIF (LIF neurons, softplus-parameterized
tau / soft-reset, surrogate-gradient spike forward = hard threshold).

Reference semantics per step (v-space, fp32):
    v   = alpha*v + (1-alpha)*x_t          # alpha = exp(-1/tau), per-neuron
    s   = (v - 1 > 0)                      # forward value of surrogate spike
    v   = v - s*r                          # soft reset, per-neuron r

Device math (P-space): Z := (v' - 1)/(1-alpha) + 1, so the threshold is the
constant 1 and the input current is RAW x (no per-neuron input scaling):
    Z_{t+1} = ((Z_t - 1) - (Z_t > 1)*C1) * C0 + x_{t+1}
    s_t     = (Z_t > 1)
with per-neuron constants C0 = alpha, C1 = 1/bprime (bprime = (1-alpha)/r).

The key trick: a single custom DVE instruction evaluates a whole GROUP of
timesteps of the recurrence via in-instruction self-feedback. Per neuron
chunk, the Z tile holds 1+KB blocks of 128 batch columns:
[Z_t0 | x_{t0+1} .. x_{t0+KB}]. The op's in0 AP covers blocks 0..KB-1 while
its out (and in1) AP covers blocks 1..KB — the DVE streams the free
dimension in order at 1 elem/cycle, so the output block written ~120 cycles
earlier is re-read as the next step's state (verified bit-exact on HW).
x is overwritten by Z in place.

Engines per group (n-major layout, 4 chunks of 128 neurons):
    DVE   : 4 chunk ops, FD = KB*128 each    (the entire recurrence)
    GPSIMD: 4 tiny boundary copies Z_tKB -> next tile block 0 (hidden)
    ACT   : spikes = Sigmoid(1e30*Z - 1e30) -> uint8, all 4 chunks at once
    PE    : completely idle
    DMA   : x in on the SP HWDGE ring (prefetched 2 groups ahead),
            spikes out on the ACT HWDGE ring

Group sizes ramp up (2,3,5,10,10,...) so the pipeline fill is one small DMA
instead of a full-size group. Steady state is HBM-bandwidth-bound
(~330 KB per step per core vs ~358 GB/s per-NeuronCore limit).

Sharding: neurons split 8 ways (512/core), batch full on every core; no
cross-core communication. Measured ~0 flipped spikes vs the fp32 CPU
reference on the full 100x128x4096 problem.
"""

import sys

import numpy as np

if "/opt/trn_rl_repo" not in sys.path:
    sys.path.insert(0, "/opt/trn_rl_repo")

T, B, N = 100, 128, 4096
NCORES = 8
NLOC = N // NCORES          # 512 neurons per core
NCH = NLOC // 128           # 4 partition-chunks of the neuron dim
BLK = 128                   # batch block width (one timestep column block)

DT = 1.0
V_TH = 1.0
TAU_MIN = 1e-3
R_MIN = 1e-6

KB = 10                     # steady-state timesteps per DVE instruction
RAMP_UP = (2, 3, 5)         # pipeline-fill group sizes
RAMP_DN = (5, 3, 2)         # pipeline-drain group sizes
PF = 2                      # x-DMA prefetch depth (groups ahead)

_NC_CACHE = {}
_LIF_OP = None


def group_sizes(n_steps):
    """Ramp-up + steady + ramp-down group sizes summing to n_steps."""
    up, dn = list(RAMP_UP), list(RAMP_DN)
    rem = n_steps - sum(up) - sum(dn)
    if rem < 0 or rem % KB:
        # fallback: uniform KB groups with a remainder group
        gs = [KB] * (n_steps // KB)
        if n_steps % KB:
            gs.append(n_steps % KB)
        return gs
    return up + [KB] * (rem // KB) + dn


def _register_lif_op():
    """Custom DVE op: out = ((in0 - 1) - (in0 > 1)*C1)*C0 + in1.

    With in0 = Z_t (prev state block), in1 = x_{t+1}, C0 = alpha (per
    partition), C1 = 1/bprime (per partition) this computes Z_{t+1}; the
    in0/out APs overlap shifted by one 128-col block so one instruction
    evaluates KB serial timesteps.
    """
    global _LIF_OP
    if _LIF_OP is not None:
        return _LIF_OP
    import concourse.dve_ops as dve_ops
    from concourse.dve_ops import DveOp, OPS, CUSTOM_DVE_SPECS, _SUB_OPCODE_FOR_NAME
    from concourse.dve_spec import Spec, Src0, Src1, C0, C1, One, lower
    from concourse.dve_uop import DveOpSpec

    name = "LIF_STREAM_ANT"
    if name in _SUB_OPCODE_FOR_NAME:
        _LIF_OP = next(op for op in OPS if op.name == name)
        return _LIF_OP

    z = Src0
    s = z > One
    spec = Spec(
        body=((z - One) - s * C1) * C0 + Src1,
        reference=lambda in0, in1, s0, s1, imm2: (
            ((in0 - 1.0) - (in0 > 1.0).astype(np.float32) * s1) * s0 + in1
        ).astype(np.float32),
    )
    row = dve_ops._CUSTOM_DVE_ROW_BASE + len(OPS)
    assert row < 0x20
    shas = {}
    for ver in ("v3", "v4"):
        tmp = DveOpSpec(name=name, opcode=row, uops=lower(spec, ver=ver),
                        rd1_en=True)
        shas[ver] = tmp.sha(ver)
    op = DveOp(name, spec, subdim=False, uops_sha=shas)
    OPS.append(op)
    CUSTOM_DVE_SPECS[name] = spec
    _SUB_OPCODE_FOR_NAME[name] = row
    _LIF_OP = op
    return op


def _build_nc(n_steps=T):
    import concourse.bacc as bacc
    import concourse.tile as tile
    from concourse import mybir

    f32 = mybir.dt.float32
    u8 = mybir.dt.uint8

    lif_op = _register_lif_op()

    nc = bacc.Bacc("TRN2", target_bir_lowering=False, debug=False)

    xf = nc.dram_tensor("xf", [128, NCH * n_steps * BLK], f32,
                        kind="ExternalInput").ap()
    z0 = nc.dram_tensor("z0", [128, NCH * BLK], f32, kind="ExternalInput").ap()
    alpha4 = nc.dram_tensor("alpha4", [128, NCH], f32, kind="ExternalInput").ap()
    rinv4 = nc.dram_tensor("rinv4", [128, NCH], f32, kind="ExternalInput").ap()
    sf = nc.dram_tensor("sf", [128, NCH * n_steps * BLK], u8,
                        kind="ExternalOutput").ap()

    _emit(nc, tile, mybir, lif_op, xf, z0, alpha4, rinv4, sf, n_steps, reps=1)

    nc.compile()
    return nc


def _emit(nc, tile, mybir, lif_op, xf, z0, alpha4, rinv4, sf, n_steps, reps=1):
    f32 = mybir.dt.float32
    u8 = mybir.dt.uint8
    from contextlib import nullcontext

    gs = group_sizes(n_steps)
    G = len(gs)
    t0s = [sum(gs[:i]) for i in range(G)]   # start step of each group
    SECMAX = (KB + 1) * BLK

    xf3 = xf.rearrange("p (c t) -> p c t", c=NCH)    # t in units of BLK cols
    sf3 = sf.rearrange("p (c t) -> p c t", c=NCH)

    with tile.TileContext(nc) as tc:
        with (
            tc.tile_pool(name="const", bufs=1) as const,
            tc.tile_pool(name="zp", bufs=PF + 3) as zpool,
            tc.tile_pool(name="sp", bufs=3) as spool,
        ):
            a_t = const.tile([128, NCH], f32)
            nc.sync.dma_start(a_t[:], alpha4)
            ri_t = const.tile([128, NCH], f32)
            nc.sync.dma_start(ri_t[:], rinv4)
            nb_t = const.tile([128, 1], f32)
            nc.gpsimd.memset(nb_t[:], -1.0e30)
            guard_t = const.tile([128, NCH], f32)

            rep_cm = tc.For_i(0, reps, 1) if reps > 1 else nullcontext()
            with rep_cm:
                zts = {}

                def alloc_and_fetch(i):
                    kb = gs[i]
                    sec = (kb + 1) * BLK
                    zt = zpool.tile([128, NCH * SECMAX], f32)
                    zts[i] = zt
                    dst = zt[:].rearrange("p (c y) -> p c y", c=NCH)[
                        :, :, BLK:sec
                    ]
                    src = xf3[:, :, t0s[i] * BLK:(t0s[i] + kb) * BLK]
                    nc.sync.dma_start(dst, src)

                # prologue: prefetch x for the first PF+1 groups, seed state
                for i in range(min(PF + 1, G)):
                    alloc_and_fetch(i)
                dst = zts[0][:].rearrange("p (c y) -> p c y", c=NCH)[:, :, :BLK]
                nc.sync.dma_start(dst, z0.rearrange("p (c b) -> p c b", c=NCH))

                for g in range(G):
                    zt = zts[g]
                    kb = gs[g]
                    sec = (kb + 1) * BLK

                    # Dependency guard: the last x block of each chunk is read
                    # only via in1, which aliases the op's own out region —
                    # Tile does not derive the DMA->op edge for it. This tiny
                    # read of one column per chunk forces the wait; the real
                    # ops follow in DVE program order.
                    nc.vector.tensor_copy(
                        guard_t[:].rearrange("p (c y) -> p c y", y=1),
                        zt[:].rearrange("p (c y) -> p c y", c=NCH)[
                            :, :, kb * BLK:kb * BLK + 1
                        ],
                    )

                    # the recurrence: one DVE op per chunk, kb steps each,
                    # in-instruction self-feedback across the kb blocks
                    for c in range(NCH):
                        base = c * SECMAX
                        nc.vector._custom_dve(
                            lif_op,
                            out=zt[:, base + BLK:base + sec],
                            in0=zt[:, base:base + kb * BLK],
                            in1=zt[:, base + BLK:base + sec],
                            s0=a_t[:, c:c + 1],
                            s1=ri_t[:, c:c + 1],
                        )

                    # state boundary into the next tile (GPSIMD: off both the
                    # DVE chain and the ACT queue, hidden under the next ops)
                    if g + 1 < G:
                        znext = zts[g + 1]
                        for c in range(NCH):
                            nc.gpsimd.tensor_copy(
                                znext[:, c * SECMAX:c * SECMAX + BLK],
                                zt[:, c * SECMAX + kb * BLK:c * SECMAX + sec],
                            )

                    # spikes: s = (Z > 1) as exact 0/1 u8
                    st = spool.tile([128, NCH * KB * BLK], u8)
                    nc.scalar.activation(
                        st[:].rearrange("p (c y) -> p c y", c=NCH)[
                            :, :, :kb * BLK
                        ],
                        zt[:].rearrange("p (c y) -> p c y", c=NCH)[
                            :, :, BLK:sec
                        ],
                        mybir.ActivationFunctionType.Sigmoid,
                        bias=nb_t[:, 0:1],
                        scale=1.0e30,
                    )
                    # spikes out on the ACT HWDGE ring (overlaps the x-in ring)
                    nc.scalar.dma_start(
                        sf3[:, :, t0s[g] * BLK:(t0s[g] + kb) * BLK],
                        st[:].rearrange("p (c y) -> p c y", c=NCH)[
                            :, :, :kb * BLK
                        ],
                    )

                    if g + PF + 1 < G:
                        alloc_and_fetch(g + PF + 1)
                    del zts[g]


def _get_nc(n_steps=T):
    if n_steps not in _NC_CACHE:
        _NC_CACHE[n_steps] = _build_nc(n_steps)
    return _NC_CACHE[n_steps]


def _derive_params(tau_raw, r_raw):
    """Per-neuron constants, fp32 softplus path matching jax CPU exactly."""
    tr = np.asarray(tau_raw, dtype=np.float32)
    rr = np.asarray(r_raw, dtype=np.float32)
    tau = np.logaddexp(np.float32(0.0), tr).astype(np.float32) + np.float32(TAU_MIN)
    alpha = np.exp(-np.float32(DT) / tau).astype(np.float32)
    r = np.logaddexp(np.float32(0.0), rr).astype(np.float32) + np.float32(R_MIN)
    beta = np.float32(1.0) - alpha
    # C1 = 1/bprime = r/beta; z0 = (1 - 1/beta)/alpha + 1 in f64 then f32
    c1 = (r.astype(np.float64) / beta.astype(np.float64)).astype(np.float32)
    z0 = (
        (1.0 - 1.0 / beta.astype(np.float64)) / alpha.astype(np.float64) + 1.0
    ).astype(np.float32)
    return alpha, c1, z0


def _core_inputs(x, alpha, c1, z0, core, n_steps):
    sl = slice(core * NLOC, (core + 1) * NLOC)
    # x[:, :, sl] is [T, B, 512]; device wants [p, (c, t, b)] flat
    xs = x[:n_steps, :, sl].reshape(n_steps, B, NCH, 128)
    xfl = np.ascontiguousarray(xs.transpose(3, 2, 0, 1), dtype=np.float32)
    xfl = xfl.reshape(128, NCH * n_steps * BLK)

    a4 = np.ascontiguousarray(alpha[sl].reshape(NCH, 128).T, dtype=np.float32)
    r4 = np.ascontiguousarray(c1[sl].reshape(NCH, 128).T, dtype=np.float32)
    z0l = z0[sl].reshape(NCH, 128).T      # [p, c]
    z0b = np.ascontiguousarray(
        np.broadcast_to(z0l[:, :, None], (128, NCH, BLK)), dtype=np.float32
    ).reshape(128, NCH * BLK)
    return {"xf": xfl, "z0": z0b, "alpha4": a4, "rinv4": r4}


def _run(x, tau_raw, r_raw, n_steps=T, trace=False, **run_kwargs):
    from concourse.bass_utils import run_bass_kernel_spmd

    alpha, c1, z0 = _derive_params(tau_raw, r_raw)
    in_maps = [
        _core_inputs(x, alpha, c1, z0, c, n_steps) for c in range(NCORES)
    ]
    nc = _get_nc(n_steps)
    res = run_bass_kernel_spmd(
        nc, in_maps, core_ids=list(range(NCORES)), trace=trace, **run_kwargs
    )
    shards = []
    for c in range(NCORES):
        sfl = res.results[c]["sf"].reshape(128, NCH, n_steps, BLK)
        # [p, c, t, b] -> [t, b, n_local = c*128 + p]
        sc = sfl.transpose(2, 3, 1, 0).reshape(n_steps, B, NLOC)
        shards.append(sc)
    out = np.concatenate(shards, axis=-1).astype(np.float32)
    return out, res


def kernel(x, tau_raw, r_raw):
    x = np.asarray(x, dtype=np.float32)
    tau_raw = np.asarray(tau_raw, dtype=np.float32)
    r_raw = np.asarray(r_raw, dtype=np.float32)
    last = None
    for attempt in range(3):
        try:
            out, _ = _run(x, tau_raw, r_raw)
            return out
        except Exception as e:  # transient NRT device errors observed rarely
            last = e
            import time as _time

            _time.sleep(2.0 * (attempt + 1))
    raise last
